# revision 17
# baseline (speedup 1.0000x reference)
"""Multi-head causal self-attention (B=2, S=2048, D=1024, H=16) on 8 trn2 cores.

Sharding: 2-way data-parallel over batch x 4-way tensor-parallel over heads.
Core c handles batch b=c//4 and heads [4*(c%4), 4*(c%4)+4).

Per-core device program (bf16 matmul operands, fp32 PSUM accumulation):
  1. QKV projections from host-pre-transposed x^T and W^T shards (all bf16).
     Q^T,K^T produced as [head-channel, token]; V as [token, channel] with a
     fused ones-column (softmax-denominator trick).
  2. Flash-style causal attention per (t-block 512, chunk of 2 heads):
     both heads' scores^T land in one [128, 2, 512] PSUM pair of banks, one
     batched exp (ScalarE, scale=1/8, bf16 out) serves both, diagonal tiles
     masked via a lower-triangle multiply on GpSimd, AV pairs trail two
     u-steps behind the scores so PE stays dense. AV accumulates into PSUM
     [65, 512] whose row 64 is the softmax denominator (ones column of V).
     Normalization after AV: denominator broadcast by a K=1 matmul,
     reciprocal_approx_fast, then one multiply into OT (bf16).
  3. o_proj partial product over this core's 256 v-dims, per t-block.
  The qkv projections for t-group g+1 and o_proj for block b-1 are
  interleaved into attention(b)'s instruction stream ("filler") so PE never
  drains while ScalarE runs exp.
  DMA: weights on the sync queue, x^T chunks on the scalar queue (parallel
  rings) so the first QK chain starts ~4us in instead of ~24us.
Host sums the 4 per-batch partials (the v-contraction all-reduce) and stacks.
"""

import numpy as np
from collections import deque
from contextlib import ExitStack

import ml_dtypes

import concourse.bass as bass
import concourse.bacc as bacc
import concourse.tile as tile
import concourse.mybir as mybir
from concourse.bass_utils import run_bass_kernel_spmd

F32 = mybir.dt.float32
BF16 = mybir.dt.bfloat16
EXP = mybir.ActivationFunctionType.Exp
NPBF = ml_dtypes.bfloat16

B, S, D = 2, 2048, 1024
H, HS = 16, 64
NCORES = 8
HPC = H // (NCORES // B)  # heads per core = 4
KD = HPC * HS             # per-core projected dims = 256
NKK = KD // 128           # head-dim partition chunks = 2
NDC = D // 128            # contraction chunks = 8
TB = 512                  # t-block width
NTB = S // TB             # 4
NUT = S // 128            # u-tiles = 16
SCALE = float(HS) ** -0.5


class Filler:
    """Queue of generators; each step(frac) emits fractional units of
    deferred PE work so filler spreads evenly over the attention loop."""

    def __init__(self, pace=1.0):
        self.gens = deque()
        self.pace = pace
        self.acc = 0.0

    def add(self, gen):
        self.gens.append(gen)

    def _one(self):
        while self.gens:
            try:
                next(self.gens[0])
                return True
            except StopIteration:
                self.gens.popleft()
        return False

    def step(self, n=None):
        self.acc += self.pace if n is None else n
        while self.acc >= 1.0:
            self.acc -= 1.0
            if not self._one():
                return

    def drain(self):
        while self._one():
            pass


def build_program():
    nc = bacc.Bacc("TRN2", target_bir_lowering=False, debug=False)
    # weights arrive pre-laid-out for SBUF ([partition, chunk, free]) so the
    # DMA is one contiguous multi-KB segment per partition line.
    xt = nc.dram_tensor("xt", [D, S], BF16, kind="ExternalInput").ap()
    wqt = nc.dram_tensor("wqt", [128, NDC, KD], BF16, kind="ExternalInput").ap()
    wkt = nc.dram_tensor("wkt", [128, NDC, KD], BF16, kind="ExternalInput").ap()
    wvt = nc.dram_tensor("wvt", [128, NDC, KD], BF16, kind="ExternalInput").ap()
    wot = nc.dram_tensor("wot", [128, NKK, D], BF16, kind="ExternalInput").ap()
    maskd = nc.dram_tensor("mask", [128, 128], BF16, kind="ExternalInput").ap()
    seld = nc.dram_tensor("sel", [2, 128], BF16, kind="ExternalInput").ap()
    y = nc.dram_tensor("y", [S, D], BF16, kind="ExternalOutput").ap()

    with tile.TileContext(nc) as tc, ExitStack() as ctx:
        wpool = ctx.enter_context(tc.tile_pool(name="w", bufs=1))
        big = ctx.enter_context(tc.tile_pool(name="big", bufs=1))
        xtg_pool = ctx.enter_context(tc.tile_pool(name="xtg", bufs=32))
        e_pool = ctx.enter_context(tc.tile_pool(name="expS", bufs=8))
        sm_pool = ctx.enter_context(tc.tile_pool(name="small", bufs=2))
        ypool = ctx.enter_context(tc.tile_pool(name="yout", bufs=3))
        psS = ctx.enter_context(tc.tile_pool(name="psS", bufs=2, space="PSUM"))
        psO = ctx.enter_context(tc.tile_pool(name="psO", bufs=3, space="PSUM"))
        psM = ctx.enter_context(tc.tile_pool(name="psM", bufs=1, space="PSUM"))

        # --- persistent SBUF tensors ---
        wq_sb = wpool.tile([128, NDC, KD], BF16)
        wk_sb = wpool.tile([128, NDC, KD], BF16)
        wv_sb = wpool.tile([128, NDC, KD], BF16)
        wo_sb = wpool.tile([128, NKK, D], BF16)
        mask_sb = wpool.tile([128, 128], BF16)
        selA_sb = wpool.tile([1, 128], BF16)
        selB_sb = wpool.tile([1, 128], BF16)
        QT = big.tile([128, NKK, S], BF16)   # [channel(2 heads), kk, token]
        KT = big.tile([128, NKK, S], BF16)
        VA = big.tile([128, HPC, NUT, HS + 1], BF16)  # [tok, head, utile, ch|1]
        OT = big.tile([128, NKK, S], BF16)   # normalized attention out^T

        # weights on the sync DMA queue (already in SBUF layout)
        nc.sync.dma_start(wq_sb[:], wqt)
        nc.sync.dma_start(wk_sb[:], wkt)
        nc.sync.dma_start(wv_sb[:], wvt)
        nc.sync.dma_start(mask_sb[:], maskd)
        nc.sync.dma_start(selA_sb[:], seld[0:1, :])
        nc.sync.dma_start(selB_sb[:], seld[1:2, :])
        nc.sync.dma_start(wo_sb[:], wot)
        nc.gpsimd.memset(VA[:, :, :, HS], 1.0)

        # x^T on the scalar DMA queue, tg-major so t-group 0 lands first;
        # batched 4 c-chunks per DMA to keep ScalarE's issue cost small.
        xts = [[None] * NTB for _ in range(NDC)]
        for tg in range(NTB):
            for h in range(2):
                t = xtg_pool.tile([128, 4, TB], BF16, tag="xtg",
                                  name=f"xt{h}_{tg}")
                nc.scalar.dma_start(
                    t[:],
                    xt[512 * h:512 * (h + 1),
                       TB * tg:TB * (tg + 1)].rearrange(
                           "(c p) t -> p c t", p=128),
                )
                for c4 in range(4):
                    xts[4 * h + c4][tg] = t[:, c4, :]

        def qkv_gen(tg):
            """Projections for t-group tg, yielding every ~4 matmuls."""
            for w_sb, dst in ((wq_sb, QT), (wk_sb, KT)):
                for kk in range(NKK):
                    ps = psM.tile([128, TB], F32, tag="m512", name="qk_ps")
                    for c in range(NDC):
                        nc.tensor.matmul(
                            ps[:],
                            w_sb[:, c, 128 * kk:128 * (kk + 1)],
                            xts[c][tg][:],
                            start=(c == 0), stop=(c == NDC - 1),
                        )
                        if c % 4 == 3:
                            yield
                    nc.vector.tensor_copy(dst[:, kk, TB * tg:TB * (tg + 1)], ps[:])
                    yield
            for tt in range(TB // 128):
                ps = psM.tile([128, TB], F32, tag="m512", name="v_ps")
                for c in range(NDC):
                    nc.tensor.matmul(
                        ps[:, 0:KD],
                        xts[c][tg][:, 128 * tt:128 * (tt + 1)],
                        wv_sb[:, c, :],
                        start=(c == 0), stop=(c == NDC - 1),
                    )
                    if c % 4 == 3:
                        yield
                ut = (TB // 128) * tg + tt
                nc.vector.tensor_copy(
                    VA[:, :, ut, 0:HS],
                    ps[:, 0:KD].rearrange("p (h c) -> p h c", c=HS),
                )
                yield

        def o_proj_gen(tb, split_engines=False):
            for i in range(4 * tb, 4 * tb + 4):
                for j in range(D // 512):
                    ps = psM.tile([128, TB], F32, tag="m512", name="yps")
                    for vc in range(NKK):
                        nc.tensor.matmul(
                            ps[:],
                            OT[:, vc, 128 * i:128 * (i + 1)],
                            wo_sb[:, vc, 512 * j:512 * (j + 1)],
                            start=(vc == 0), stop=(vc == NKK - 1),
                        )
                    yt = ypool.tile([128, TB], BF16, tag="yt")
                    # tail block: drain copies on both engines in parallel
                    if split_engines and (i + j) % 2 == 1:
                        nc.scalar.copy(yt[:], ps[:])
                        nc.scalar.dma_start(
                            y[128 * i:128 * (i + 1), 512 * j:512 * (j + 1)],
                            yt[:])
                    else:
                        nc.vector.tensor_copy(yt[:], ps[:])
                        nc.sync.dma_start(
                            y[128 * i:128 * (i + 1), 512 * j:512 * (j + 1)],
                            yt[:])
                    yield

        def attention(tb, filler):
            """Causal attention for t-block tb, heads paired per chunk, AV
            trailing two u-steps behind scores. filler work is pulled in
            after each u-step so PE stays dense while ScalarE runs exp."""
            nut = 4 * tb + 4
            for hp in range(NKK):
                Os = [psO.tile([HS + 1, TB], F32, tag="av", name=f"O{g}")
                      for g in range(2)]

                def av_pair(pes, ptoff, pk, stop):
                    for g in range(2):
                        nc.tensor.matmul(
                            Os[g][:, ptoff:],
                            VA[:, 2 * hp + g, pk, :],
                            pes[:, g, ptoff:],
                            start=(pk == 0), stop=stop,
                        )

                pend = []
                for k in range(nut):
                    toff = max(0, 128 * (k - 4 * tb))
                    sp = psS.tile([128, 2, TB], F32, tag="sp")
                    for g in range(2):
                        nc.tensor.matmul(
                            sp[:, g, toff:],
                            KT[64 * g:64 * g + 64, hp, 128 * k:128 * (k + 1)],
                            QT[64 * g:64 * g + 64, hp,
                               TB * tb + toff:TB * (tb + 1)],
                            start=True, stop=True,
                        )
                    es = e_pool.tile([128, 2, TB], BF16, tag="expS")
                    nc.scalar.activation(es[:, :, toff:], sp[:, :, toff:], EXP,
                                         scale=SCALE)
                    if k >= 4 * tb:  # diagonal: zero the u>t triangle
                        for g in range(2):
                            nc.gpsimd.tensor_mul(
                                es[:, g, toff:toff + 128],
                                es[:, g, toff:toff + 128], mask_sb[:]
                            )
                    pend.append((es, toff, k))
                    if len(pend) > 2:
                        av_pair(*pend.pop(0), stop=False)
                    filler.step()
                for i, p in enumerate(pend):
                    av_pair(*p, stop=(i == len(pend) - 1))
                dens = [sm_pool.tile([1, TB], BF16, tag=f"den{g}",
                                     name=f"den{g}") for g in range(2)]
                for g in range(2):
                    nc.vector.tensor_copy(dens[g][:], Os[g][HS:HS + 1, :])
                bc_ps = psM.tile([128, TB], F32, tag="m512", name="bc_ps")
                nc.tensor.matmul(bc_ps[:], selA_sb[:], dens[0][:],
                                 start=True, stop=False)
                nc.tensor.matmul(bc_ps[:], selB_sb[:], dens[1][:],
                                 start=False, stop=True)
                # keep PE fed while the norm chain drains on DVE
                filler.step(2)
                bc = sm_pool.tile([128, TB], F32, tag="bc_sb")
                nc.vector.reciprocal_approx_fast(bc[:], bc_ps[:])
                for g in range(2):
                    ro = 64 * g
                    nc.vector.tensor_mul(
                        OT[ro:ro + 64, hp, TB * tb:TB * (tb + 1)],
                        Os[g][0:HS, :], bc[ro:ro + 64, :]
                    )
                filler.step(2)

        boot = Filler()
        boot.add(qkv_gen(0))
        boot.drain()
        # filler content per attention block, paced to spread over its
        # k-steps: o_proj work is deferred toward the late (exp-paced) blocks.
        plans = [
            (1.5, [qkv_gen(1)]),
            (0.75, [qkv_gen(2)]),
            (0.5, [qkv_gen(3)]),
            (0.375, [o_proj_gen(0), o_proj_gen(1), o_proj_gen(2)]),
        ]
        for tb in range(NTB):
            pace, gens = plans[tb]
            f = Filler(pace)
            for g in gens:
                f.add(g)
            attention(tb, f)
            f.drain()
        tail = Filler()
        tail.add(o_proj_gen(NTB - 1, split_engines=True))
        tail.drain()

    nc.compile()
    return nc


def make_in_maps(x, q_w, k_w, v_w, o_w):
    x = np.asarray(x, dtype=np.float32)
    mask = np.triu(np.ones((128, 128), np.float32)).astype(NPBF)
    sel = np.zeros((2, 128), dtype=np.float32)
    sel[0, 0:64] = 1.0
    sel[1, 64:128] = 1.0
    sel = sel.astype(NPBF)
    xtb = [np.ascontiguousarray(x[b].T).astype(NPBF) for b in range(B)]

    def sbuf_layout(w_t, nchunk):
        # [nchunk*128, F] -> [128, nchunk, F] so each partition line is one
        # contiguous DMA segment
        f = w_t.shape[1]
        return np.ascontiguousarray(
            w_t.reshape(nchunk, 128, f).transpose(1, 0, 2)).astype(NPBF)

    in_maps = []
    for c in range(NCORES):
        b, hg = divmod(c, NCORES // B)
        sl = slice(hg * KD, (hg + 1) * KD)
        in_maps.append({
            "xt": xtb[b],
            "wqt": sbuf_layout(np.asarray(q_w, np.float32)[sl, :].T, NDC),
            "wkt": sbuf_layout(np.asarray(k_w, np.float32)[sl, :].T, NDC),
            "wvt": sbuf_layout(np.asarray(v_w, np.float32)[sl, :].T, NDC),
            "wot": sbuf_layout(np.asarray(o_w, np.float32)[:, sl].T, NKK),
            "mask": mask,
            "sel": sel,
        })
    return in_maps


def combine_outputs(results):
    """results: list of 8 dicts with per-core partial y [S, D]."""
    per_b = NCORES // B
    ys = [np.asarray(results[c]["y"]).astype(np.float32) for c in range(NCORES)]
    out = np.stack(
        [sum(ys[b * per_b + i] for i in range(per_b)) for b in range(B)]
    )
    return np.ascontiguousarray(out, dtype=np.float32)


_PROGRAM = None


def kernel(x, q_proj_weight, k_proj_weight, v_proj_weight, o_proj_weight,
           **extra):
    global _PROGRAM
    if _PROGRAM is None:
        _PROGRAM = build_program()
    in_maps = make_in_maps(x, q_proj_weight, k_proj_weight, v_proj_weight,
                           o_proj_weight)
    res = run_bass_kernel_spmd(_PROGRAM, in_maps, list(range(NCORES)))
    return combine_outputs(res.results)


if __name__ == "__main__":
    nc = build_program()
    print("program built")


# revision 19
# speedup vs baseline: 1.0660x; 1.0660x over previous
"""Multi-head causal self-attention (B=2, S=2048, D=1024, H=16) on 8 trn2 cores.

Sharding: 2-way data-parallel over batch x 4-way tensor-parallel over heads.
Core c handles batch b=c//4 and heads [4*(c%4), 4*(c%4)+4).

Per-core device program (bf16 matmul operands, fp32 PSUM accumulation):
  1. QKV projections from host-pre-transposed x^T and W^T shards (all bf16).
     Q^T,K^T produced as [head-channel, token]; V as [token, channel] with a
     fused ones-column (softmax-denominator trick).
  2. Flash-style causal attention per (t-block 512, chunk of 2 heads):
     both heads' scores^T land in one [128, 2, 512] PSUM pair of banks, one
     batched exp (ScalarE, scale=1/8, bf16 out) serves both, diagonal tiles
     masked via a lower-triangle multiply on GpSimd, AV pairs trail two
     u-steps behind the scores so PE stays dense. AV accumulates into PSUM
     [65, 512] whose row 64 is the softmax denominator (ones column of V).
     Normalization after AV: denominator broadcast by a K=1 matmul,
     reciprocal_approx_fast, then one multiply into OT (bf16).
  3. o_proj partial product over this core's 256 v-dims, per t-block.
  The qkv projections for t-group g+1 and o_proj for block b-1 are
  interleaved into attention(b)'s instruction stream ("filler") so PE never
  drains while ScalarE runs exp.
  DMA: weights on the sync queue, x^T chunks on the scalar queue (parallel
  rings) so the first QK chain starts ~4us in instead of ~24us.
Host sums the 4 per-batch partials (the v-contraction all-reduce) and stacks.
"""

import numpy as np
from collections import deque
from contextlib import ExitStack

import ml_dtypes

import concourse.bass as bass
import concourse.bacc as bacc
import concourse.tile as tile
import concourse.mybir as mybir
from concourse.bass_utils import run_bass_kernel_spmd

F32 = mybir.dt.float32
BF16 = mybir.dt.bfloat16
EXP = mybir.ActivationFunctionType.Exp
NPBF = ml_dtypes.bfloat16

B, S, D = 2, 2048, 1024
H, HS = 16, 64
NCORES = 8
HPC = H // (NCORES // B)  # heads per core = 4
KD = HPC * HS             # per-core projected dims = 256
NKK = KD // 128           # head-dim partition chunks = 2
NDC = D // 128            # contraction chunks = 8
TB = 512                  # t-block width
NTB = S // TB             # 4
NUT = S // 128            # u-tiles = 16
SCALE = float(HS) ** -0.5


class Filler:
    """Queue of generators; each step(frac) emits fractional units of
    deferred PE work so filler spreads evenly over the attention loop."""

    def __init__(self, pace=1.0):
        self.gens = deque()
        self.pace = pace
        self.acc = 0.0

    def add(self, gen):
        self.gens.append(gen)

    def _one(self):
        while self.gens:
            try:
                next(self.gens[0])
                return True
            except StopIteration:
                self.gens.popleft()
        return False

    def step(self, n=None):
        self.acc += self.pace if n is None else n
        while self.acc >= 1.0:
            self.acc -= 1.0
            if not self._one():
                return

    def drain(self):
        while self._one():
            pass


def build_program():
    nc = bacc.Bacc("TRN2", target_bir_lowering=False, debug=False)
    # weights arrive pre-laid-out for SBUF ([partition, chunk, free]) so the
    # DMA is one contiguous multi-KB segment per partition line.
    xt = nc.dram_tensor("xt", [D, S], BF16, kind="ExternalInput").ap()
    wqt = nc.dram_tensor("wqt", [128, NKK, NDC, 128], BF16, kind="ExternalInput").ap()
    wkt = nc.dram_tensor("wkt", [128, NDC, KD], BF16, kind="ExternalInput").ap()
    wvt = nc.dram_tensor("wvt", [128, NDC, KD], BF16, kind="ExternalInput").ap()
    wot = nc.dram_tensor("wot", [128, NKK, D], BF16, kind="ExternalInput").ap()
    maskd = nc.dram_tensor("mask", [128, 128], BF16, kind="ExternalInput").ap()
    seld = nc.dram_tensor("sel", [2, 128], BF16, kind="ExternalInput").ap()
    y = nc.dram_tensor("y", [S, D], BF16, kind="ExternalOutput").ap()

    with tile.TileContext(nc) as tc, ExitStack() as ctx:
        wpool = ctx.enter_context(tc.tile_pool(name="w", bufs=1))
        big = ctx.enter_context(tc.tile_pool(name="big", bufs=1))
        xtg_pool = ctx.enter_context(tc.tile_pool(name="xtg", bufs=32))
        e_pool = ctx.enter_context(tc.tile_pool(name="expS", bufs=6))
        sm_pool = ctx.enter_context(tc.tile_pool(name="small", bufs=2))
        ypool = ctx.enter_context(tc.tile_pool(name="yout", bufs=3))
        psS = ctx.enter_context(tc.tile_pool(name="psS", bufs=2, space="PSUM"))
        psO = ctx.enter_context(tc.tile_pool(name="psO", bufs=2, space="PSUM"))
        psM = ctx.enter_context(tc.tile_pool(name="psM", bufs=2, space="PSUM"))

        # --- persistent SBUF tensors ---
        wq_sb = wpool.tile([128, NDC, KD], BF16)
        wk_sb = wpool.tile([128, NDC, KD], BF16)
        wv_sb = wpool.tile([128, NDC, KD], BF16)
        wo_sb = wpool.tile([128, NKK, D], BF16)
        mask_sb = wpool.tile([128, 128], BF16)
        selA_sb = wpool.tile([1, 128], BF16)
        selB_sb = wpool.tile([1, 128], BF16)
        QT = big.tile([128, NKK, S], BF16)   # [channel(2 heads), kk, token]
        KT = big.tile([128, NKK, S], BF16)
        VA = big.tile([128, HPC, NUT, HS + 1], BF16)  # [tok, head, utile, ch|1]
        OT = big.tile([128, NKK, S], BF16)   # normalized attention out^T

        # weights on the sync DMA queue (already in SBUF layout); wq split
        # into kk-halves so the first QK chain is gated on 0.25MB, not 0.5MB
        for kk in range(NKK):
            nc.sync.dma_start(wq_sb[:, :, 128 * kk:128 * (kk + 1)], wqt[:, kk])
        nc.sync.dma_start(wk_sb[:], wkt)
        nc.sync.dma_start(wv_sb[:], wvt)
        nc.sync.dma_start(mask_sb[:], maskd)
        nc.sync.dma_start(selA_sb[:], seld[0:1, :])
        nc.sync.dma_start(selB_sb[:], seld[1:2, :])
        nc.sync.dma_start(wo_sb[:], wot)
        nc.gpsimd.memset(VA[:, :, :, HS], 1.0)

        # x^T on the scalar DMA queue, tg-major so t-group 0 lands first;
        # batched 4 c-chunks per DMA to keep ScalarE's issue cost small.
        xts = [[None] * NTB for _ in range(NDC)]
        for tg in range(NTB):
            for h in range(2):
                t = xtg_pool.tile([128, 4, TB], BF16, tag="xtg",
                                  name=f"xt{h}_{tg}")
                nc.scalar.dma_start(
                    t[:],
                    xt[512 * h:512 * (h + 1),
                       TB * tg:TB * (tg + 1)].rearrange(
                           "(c p) t -> p c t", p=128),
                )
                for c4 in range(4):
                    xts[4 * h + c4][tg] = t[:, c4, :]

        def qkv_gen(tg):
            """Projections for t-group tg, yielding every ~4 matmuls."""
            for w_sb, dst in ((wq_sb, QT), (wk_sb, KT)):
                for kk in range(NKK):
                    ps = psM.tile([128, TB], F32, tag="m512", name="qk_ps")
                    for c in range(NDC):
                        nc.tensor.matmul(
                            ps[:],
                            w_sb[:, c, 128 * kk:128 * (kk + 1)],
                            xts[c][tg][:],
                            start=(c == 0), stop=(c == NDC - 1),
                        )
                        if c % 4 == 3:
                            yield
                    nc.vector.tensor_copy(dst[:, kk, TB * tg:TB * (tg + 1)], ps[:])
                    yield
            for tt in range(TB // 128):
                ps = psM.tile([128, TB], F32, tag="m512", name="v_ps")
                for c in range(NDC):
                    nc.tensor.matmul(
                        ps[:, 0:KD],
                        xts[c][tg][:, 128 * tt:128 * (tt + 1)],
                        wv_sb[:, c, :],
                        start=(c == 0), stop=(c == NDC - 1),
                    )
                    if c % 4 == 3:
                        yield
                ut = (TB // 128) * tg + tt
                nc.vector.tensor_copy(
                    VA[:, :, ut, 0:HS],
                    ps[:, 0:KD].rearrange("p (h c) -> p h c", c=HS),
                )
                yield

        def o_proj_gen(tb, split_engines=False):
            for i in range(4 * tb, 4 * tb + 4):
                for j in range(D // 512):
                    ps = psM.tile([128, TB], F32, tag="m512", name="yps")
                    for vc in range(NKK):
                        nc.tensor.matmul(
                            ps[:],
                            OT[:, vc, 128 * i:128 * (i + 1)],
                            wo_sb[:, vc, 512 * j:512 * (j + 1)],
                            start=(vc == 0), stop=(vc == NKK - 1),
                        )
                    yt = ypool.tile([128, TB], BF16, tag="yt")
                    # tail block: drain copies on both engines in parallel
                    if split_engines and (i + j) % 2 == 1:
                        nc.scalar.copy(yt[:], ps[:])
                        nc.scalar.dma_start(
                            y[128 * i:128 * (i + 1), 512 * j:512 * (j + 1)],
                            yt[:])
                    else:
                        nc.vector.tensor_copy(yt[:], ps[:])
                        nc.sync.dma_start(
                            y[128 * i:128 * (i + 1), 512 * j:512 * (j + 1)],
                            yt[:])
                    yield

        def attention(tb, filler):
            """Causal attention for t-block tb, heads paired per chunk, AV
            trailing two u-steps behind scores. filler work is pulled in
            after each u-step so PE stays dense while ScalarE runs exp."""
            nut = 4 * tb + 4
            for hp in range(NKK):
                Os = [psO.tile([HS + 1, TB], F32, tag="av", name=f"O{g}")
                      for g in range(2)]

                def av_pair(pes, ptoff, pk, stop):
                    for g in range(2):
                        nc.tensor.matmul(
                            Os[g][:, ptoff:],
                            VA[:, 2 * hp + g, pk, :],
                            pes[:, g, ptoff:],
                            start=(pk == 0), stop=stop,
                        )

                pend = []
                for k in range(nut):
                    toff = max(0, 128 * (k - 4 * tb))
                    sp = psS.tile([128, 2, TB], F32, tag="sp")
                    for g in range(2):
                        nc.tensor.matmul(
                            sp[:, g, toff:],
                            KT[64 * g:64 * g + 64, hp, 128 * k:128 * (k + 1)],
                            QT[64 * g:64 * g + 64, hp,
                               TB * tb + toff:TB * (tb + 1)],
                            start=True, stop=True,
                        )
                    es = e_pool.tile([128, 2, TB], BF16, tag="expS")
                    nc.scalar.activation(es[:, :, toff:], sp[:, :, toff:], EXP,
                                         scale=SCALE)
                    if k >= 4 * tb:  # diagonal: zero the u>t triangle
                        for g in range(2):
                            nc.gpsimd.tensor_mul(
                                es[:, g, toff:toff + 128],
                                es[:, g, toff:toff + 128], mask_sb[:]
                            )
                    pend.append((es, toff, k))
                    if len(pend) > 2:
                        av_pair(*pend.pop(0), stop=False)
                    filler.step()
                for i, p in enumerate(pend):
                    av_pair(*p, stop=(i == len(pend) - 1))
                dens = [sm_pool.tile([1, TB], BF16, tag=f"den{g}",
                                     name=f"den{g}") for g in range(2)]
                for g in range(2):
                    nc.scalar.copy(dens[g][:], Os[g][HS:HS + 1, :])
                bc_ps = psM.tile([128, TB], F32, tag="m512", name="bc_ps")
                nc.tensor.matmul(bc_ps[:], selA_sb[:], dens[0][:],
                                 start=True, stop=False)
                nc.tensor.matmul(bc_ps[:], selB_sb[:], dens[1][:],
                                 start=False, stop=True)
                # keep PE fed while the norm chain drains on DVE
                filler.step(2)
                bc = sm_pool.tile([128, TB], F32, tag="bc_sb")
                nc.vector.reciprocal_approx_fast(bc[:], bc_ps[:])
                for g in range(2):
                    ro = 64 * g
                    nc.vector.tensor_mul(
                        OT[ro:ro + 64, hp, TB * tb:TB * (tb + 1)],
                        Os[g][0:HS, :], bc[ro:ro + 64, :]
                    )
                filler.step(2)

        boot = Filler()
        boot.add(qkv_gen(0))
        boot.drain()
        # filler content per attention block, paced to spread over its
        # k-steps: o_proj work is deferred toward the late (exp-paced) blocks.
        plans = [
            (1.5, [qkv_gen(1)]),
            (0.75, [qkv_gen(2)]),
            (0.5, [qkv_gen(3)]),
            (0.375, [o_proj_gen(0), o_proj_gen(1), o_proj_gen(2)]),
        ]
        for tb in range(NTB):
            pace, gens = plans[tb]
            f = Filler(pace)
            for g in gens:
                f.add(g)
            attention(tb, f)
            f.drain()
        tail = Filler()
        tail.add(o_proj_gen(NTB - 1, split_engines=True))
        tail.drain()

    nc.compile()
    return nc


def make_in_maps(x, q_w, k_w, v_w, o_w):
    x = np.asarray(x, dtype=np.float32)
    mask = np.triu(np.ones((128, 128), np.float32)).astype(NPBF)
    sel = np.zeros((2, 128), dtype=np.float32)
    sel[0, 0:64] = 1.0
    sel[1, 64:128] = 1.0
    sel = sel.astype(NPBF)
    xtb = [np.ascontiguousarray(x[b].T).astype(NPBF) for b in range(B)]

    def sbuf_layout(w_t, nchunk):
        # [nchunk*128, F] -> [128, nchunk, F] so each partition line is one
        # contiguous DMA segment
        f = w_t.shape[1]
        return np.ascontiguousarray(
            w_t.reshape(nchunk, 128, f).transpose(1, 0, 2)).astype(NPBF)

    in_maps = []
    for c in range(NCORES):
        b, hg = divmod(c, NCORES // B)
        sl = slice(hg * KD, (hg + 1) * KD)
        in_maps.append({
            "xt": xtb[b],
            "wqt": np.ascontiguousarray(
                sbuf_layout(np.asarray(q_w, np.float32)[sl, :].T, NDC)
                .reshape(128, NDC, NKK, 128).transpose(0, 2, 1, 3)),
            "wkt": sbuf_layout(np.asarray(k_w, np.float32)[sl, :].T, NDC),
            "wvt": sbuf_layout(np.asarray(v_w, np.float32)[sl, :].T, NDC),
            "wot": sbuf_layout(np.asarray(o_w, np.float32)[:, sl].T, NKK),
            "mask": mask,
            "sel": sel,
        })
    return in_maps


def combine_outputs(results):
    """results: list of 8 dicts with per-core partial y [S, D]."""
    per_b = NCORES // B
    ys = [np.asarray(results[c]["y"]).astype(np.float32) for c in range(NCORES)]
    out = np.stack(
        [sum(ys[b * per_b + i] for i in range(per_b)) for b in range(B)]
    )
    return np.ascontiguousarray(out, dtype=np.float32)


_PROGRAM = None


def kernel(x, q_proj_weight, k_proj_weight, v_proj_weight, o_proj_weight,
           **extra):
    global _PROGRAM
    if _PROGRAM is None:
        _PROGRAM = build_program()
    in_maps = make_in_maps(x, q_proj_weight, k_proj_weight, v_proj_weight,
                           o_proj_weight)
    res = run_bass_kernel_spmd(_PROGRAM, in_maps, list(range(NCORES)))
    return combine_outputs(res.results)


if __name__ == "__main__":
    nc = build_program()
    print("program built")


# revision 20
# speedup vs baseline: 1.1009x; 1.0328x over previous
"""Multi-head causal self-attention (B=2, S=2048, D=1024, H=16) on 8 trn2 cores.

Sharding: 2-way data-parallel over batch x 4-way tensor-parallel over heads.
Core c handles batch b=c//4 and heads [4*(c%4), 4*(c%4)+4).

Per-core device program (bf16 matmul operands, fp32 PSUM accumulation):
  1. QKV projections from host-pre-transposed x^T and W^T shards (all bf16).
     Q^T,K^T produced as [head-channel, token]; V as [token, channel] with a
     fused ones-column (softmax-denominator trick).
  2. Flash-style causal attention per (t-block 512, chunk of 2 heads):
     both heads' scores^T land in one [128, 2, 512] PSUM pair of banks, one
     batched exp (ScalarE, scale=1/8, bf16 out) serves both, diagonal tiles
     masked via a lower-triangle multiply on GpSimd, AV pairs trail two
     u-steps behind the scores so PE stays dense. AV accumulates into PSUM
     [65, 512] whose row 64 is the softmax denominator (ones column of V).
     Normalization after AV: denominator broadcast by a K=1 matmul,
     reciprocal_approx_fast, then one multiply into OT (bf16).
  3. o_proj partial product over this core's 256 v-dims, per t-block.
  The qkv projections for t-group g+1 and o_proj for block b-1 are
  interleaved into attention(b)'s instruction stream ("filler") so PE never
  drains while ScalarE runs exp.
  DMA: weights on the sync queue, x^T chunks on the scalar queue (parallel
  rings) so the first QK chain starts ~4us in instead of ~24us.
Host sums the 4 per-batch partials (the v-contraction all-reduce) and stacks.
"""

import numpy as np
from collections import deque
from contextlib import ExitStack

import ml_dtypes

import concourse.bass as bass
import concourse.bacc as bacc
import concourse.tile as tile
import concourse.mybir as mybir
from concourse.bass_utils import run_bass_kernel_spmd

F32 = mybir.dt.float32
BF16 = mybir.dt.bfloat16
EXP = mybir.ActivationFunctionType.Exp
NPBF = ml_dtypes.bfloat16

B, S, D = 2, 2048, 1024
H, HS = 16, 64
NCORES = 8
HPC = H // (NCORES // B)  # heads per core = 4
KD = HPC * HS             # per-core projected dims = 256
NKK = KD // 128           # head-dim partition chunks = 2
NDC = D // 128            # contraction chunks = 8
TB = 512                  # t-block width
NTB = S // TB             # 4
NUT = S // 128            # u-tiles = 16
SCALE = float(HS) ** -0.5


class Filler:
    """Queue of generators; each step(frac) emits fractional units of
    deferred PE work so filler spreads evenly over the attention loop."""

    def __init__(self, pace=1.0):
        self.gens = deque()
        self.pace = pace
        self.acc = 0.0

    def add(self, gen):
        self.gens.append(gen)

    def _one(self):
        while self.gens:
            try:
                next(self.gens[0])
                return True
            except StopIteration:
                self.gens.popleft()
        return False

    def step(self, n=None):
        self.acc += self.pace if n is None else n
        while self.acc >= 1.0:
            self.acc -= 1.0
            if not self._one():
                return

    def drain(self):
        while self._one():
            pass


def build_program():
    nc = bacc.Bacc("TRN2", target_bir_lowering=False, debug=False)
    # weights arrive pre-laid-out for SBUF ([partition, chunk, free]) so the
    # DMA is one contiguous multi-KB segment per partition line.
    xt = nc.dram_tensor("xt", [D, S], BF16, kind="ExternalInput").ap()
    wqt = nc.dram_tensor("wqt", [128, NKK, NDC, 128], BF16, kind="ExternalInput").ap()
    wkt = nc.dram_tensor("wkt", [128, NDC, KD], BF16, kind="ExternalInput").ap()
    wvt = nc.dram_tensor("wvt", [128, NDC, KD], BF16, kind="ExternalInput").ap()
    wot = nc.dram_tensor("wot", [128, NKK, D], BF16, kind="ExternalInput").ap()
    maskd = nc.dram_tensor("mask", [128, 128], BF16, kind="ExternalInput").ap()
    seld = nc.dram_tensor("sel", [2, 128], BF16, kind="ExternalInput").ap()
    y = nc.dram_tensor("y", [S, D], BF16, kind="ExternalOutput").ap()

    with tile.TileContext(nc) as tc, ExitStack() as ctx:
        wpool = ctx.enter_context(tc.tile_pool(name="w", bufs=1))
        big = ctx.enter_context(tc.tile_pool(name="big", bufs=1))
        xtg_pool = ctx.enter_context(tc.tile_pool(name="xtg", bufs=32))
        e_pool = ctx.enter_context(tc.tile_pool(name="expS", bufs=6))
        sm_pool = ctx.enter_context(tc.tile_pool(name="small", bufs=2))
        ypool = ctx.enter_context(tc.tile_pool(name="yout", bufs=3))
        psS = ctx.enter_context(tc.tile_pool(name="psS", bufs=2, space="PSUM"))
        psO = ctx.enter_context(tc.tile_pool(name="psO", bufs=2, space="PSUM"))
        psM = ctx.enter_context(tc.tile_pool(name="psM", bufs=2, space="PSUM"))

        # --- persistent SBUF tensors ---
        wq_sb = wpool.tile([128, NDC, KD], BF16)
        wk_sb = wpool.tile([128, NDC, KD], BF16)
        wv_sb = wpool.tile([128, NDC, KD], BF16)
        wo_sb = wpool.tile([128, NKK, D], BF16)
        mask_sb = wpool.tile([128, 128], BF16)
        selA_sb = wpool.tile([1, 128], BF16)
        selB_sb = wpool.tile([1, 128], BF16)
        QT = big.tile([128, NKK, S], BF16)   # [channel(2 heads), kk, token]
        KT = big.tile([128, NKK, S], BF16)
        VA = big.tile([128, HPC, NUT, HS + 1], BF16)  # [tok, head, utile, ch|1]
        OT = big.tile([128, NKK, S], BF16)   # normalized attention out^T

        # weights on the sync DMA queue (already in SBUF layout); wq split
        # into kk-halves so the first QK chain is gated on 0.25MB, not 0.5MB
        for kk in range(NKK):
            nc.sync.dma_start(wq_sb[:, :, 128 * kk:128 * (kk + 1)], wqt[:, kk])
        nc.sync.dma_start(wk_sb[:], wkt)
        nc.sync.dma_start(wv_sb[:], wvt)
        nc.sync.dma_start(mask_sb[:], maskd)
        nc.sync.dma_start(selA_sb[:], seld[0:1, :])
        nc.sync.dma_start(selB_sb[:], seld[1:2, :])
        nc.sync.dma_start(wo_sb[:], wot)
        nc.gpsimd.memset(VA[:, :, :, HS], 1.0)

        # x^T on the scalar DMA queue, tg-major so t-group 0 lands first;
        # batched 4 c-chunks per DMA to keep ScalarE's issue cost small.
        xts = [[None] * NTB for _ in range(NDC)]
        for tg in range(NTB):
            for h in range(2):
                t = xtg_pool.tile([128, 4, TB], BF16, tag="xtg",
                                  name=f"xt{h}_{tg}")
                nc.scalar.dma_start(
                    t[:],
                    xt[512 * h:512 * (h + 1),
                       TB * tg:TB * (tg + 1)].rearrange(
                           "(c p) t -> p c t", p=128),
                )
                for c4 in range(4):
                    xts[4 * h + c4][tg] = t[:, c4, :]

        def qkv_gen(tg):
            """Projections for t-group tg, yielding every ~4 matmuls."""
            for w_sb, dst in ((wq_sb, QT), (wk_sb, KT)):
                for kk in range(NKK):
                    ps = psM.tile([128, TB], F32, tag="m512", name="qk_ps")
                    for c in range(NDC):
                        nc.tensor.matmul(
                            ps[:],
                            w_sb[:, c, 128 * kk:128 * (kk + 1)],
                            xts[c][tg][:],
                            start=(c == 0), stop=(c == NDC - 1),
                        )
                        if c % 4 == 3:
                            yield
                    nc.vector.tensor_copy(dst[:, kk, TB * tg:TB * (tg + 1)], ps[:])
                    yield
            for tt in range(TB // 128):
                ps = psM.tile([128, TB], F32, tag="m512", name="v_ps")
                for c in range(NDC):
                    nc.tensor.matmul(
                        ps[:, 0:KD],
                        xts[c][tg][:, 128 * tt:128 * (tt + 1)],
                        wv_sb[:, c, :],
                        start=(c == 0), stop=(c == NDC - 1),
                    )
                    if c % 4 == 3:
                        yield
                ut = (TB // 128) * tg + tt
                nc.vector.tensor_copy(
                    VA[:, :, ut, 0:HS],
                    ps[:, 0:KD].rearrange("p (h c) -> p h c", c=HS),
                )
                yield

        def o_proj_gen(tb, split_engines=False):
            for i in range(4 * tb, 4 * tb + 4):
                for j in range(D // 512):
                    ps = psM.tile([128, TB], F32, tag="m512", name="yps")
                    for vc in range(NKK):
                        nc.tensor.matmul(
                            ps[:],
                            OT[:, vc, 128 * i:128 * (i + 1)],
                            wo_sb[:, vc, 512 * j:512 * (j + 1)],
                            start=(vc == 0), stop=(vc == NKK - 1),
                        )
                    yt = ypool.tile([128, TB], BF16, tag="yt")
                    # tail block: drain copies on both engines in parallel
                    if split_engines and (i + j) % 2 == 1:
                        nc.scalar.copy(yt[:], ps[:])
                        nc.scalar.dma_start(
                            y[128 * i:128 * (i + 1), 512 * j:512 * (j + 1)],
                            yt[:])
                    else:
                        nc.vector.tensor_copy(yt[:], ps[:])
                        nc.sync.dma_start(
                            y[128 * i:128 * (i + 1), 512 * j:512 * (j + 1)],
                            yt[:])
                    yield

        def attention(tb, filler):
            """Causal attention for t-block tb, heads paired per chunk, AV
            trailing two u-steps behind scores. filler work is pulled in
            after each u-step so PE stays dense while ScalarE runs exp."""
            nut = 4 * tb + 4
            for hp in range(NKK):
                Os = [psO.tile([HS + 1, TB], F32, tag="av", name=f"O{g}")
                      for g in range(2)]

                def av_pair(pes, ptoff, pk, stop):
                    for g in range(2):
                        nc.tensor.matmul(
                            Os[g][:, ptoff:],
                            VA[:, 2 * hp + g, pk, :],
                            pes[:, g, ptoff:],
                            start=(pk == 0), stop=stop,
                        )

                pend = []
                for k in range(nut):
                    toff = max(0, 128 * (k - 4 * tb))
                    sp = psS.tile([128, 2, TB], F32, tag="sp")
                    for g in range(2):
                        nc.tensor.matmul(
                            sp[:, g, toff:],
                            KT[64 * g:64 * g + 64, hp, 128 * k:128 * (k + 1)],
                            QT[64 * g:64 * g + 64, hp,
                               TB * tb + toff:TB * (tb + 1)],
                            start=True, stop=True,
                        )
                    es = e_pool.tile([128, 2, TB], BF16, tag="expS")
                    nc.scalar.activation(es[:, :, toff:], sp[:, :, toff:], EXP,
                                         scale=SCALE)
                    if k >= 4 * tb:  # diagonal: zero the u>t triangle
                        for g in range(2):
                            nc.gpsimd.tensor_mul(
                                es[:, g, toff:toff + 128],
                                es[:, g, toff:toff + 128], mask_sb[:]
                            )
                    pend.append((es, toff, k))
                    if len(pend) > 2:
                        av_pair(*pend.pop(0), stop=False)
                    filler.step()
                for i, p in enumerate(pend):
                    av_pair(*p, stop=(i == len(pend) - 1))
                dens = [sm_pool.tile([1, TB], BF16, tag=f"den{g}",
                                     name=f"den{g}") for g in range(2)]
                for g in range(2):
                    nc.vector.tensor_copy(dens[g][:], Os[g][HS:HS + 1, :])
                bc_ps = psM.tile([128, TB], F32, tag="m512", name="bc_ps")
                nc.tensor.matmul(bc_ps[:], selA_sb[:], dens[0][:],
                                 start=True, stop=False)
                nc.tensor.matmul(bc_ps[:], selB_sb[:], dens[1][:],
                                 start=False, stop=True)
                # keep PE fed while the norm chain drains on DVE
                filler.step(2)
                bc = sm_pool.tile([128, TB], F32, tag="bc_sb")
                nc.vector.reciprocal_approx_fast(bc[:], bc_ps[:])
                for g in range(2):
                    ro = 64 * g
                    nc.vector.tensor_mul(
                        OT[ro:ro + 64, hp, TB * tb:TB * (tb + 1)],
                        Os[g][0:HS, :], bc[ro:ro + 64, :]
                    )
                filler.step(2)

        boot = Filler()
        boot.add(qkv_gen(0))
        boot.drain()
        # filler content per attention block, paced to spread over its
        # k-steps: o_proj work is deferred toward the late (exp-paced) blocks.
        plans = [
            (1.5, [qkv_gen(1)]),
            (0.75, [qkv_gen(2)]),
            (0.5, [qkv_gen(3)]),
            (0.375, [o_proj_gen(0), o_proj_gen(1), o_proj_gen(2)]),
        ]
        for tb in range(NTB):
            pace, gens = plans[tb]
            f = Filler(pace)
            for g in gens:
                f.add(g)
            attention(tb, f)
            f.drain()
        tail = Filler()
        tail.add(o_proj_gen(NTB - 1, split_engines=True))
        tail.drain()

    nc.compile()
    return nc


def make_in_maps(x, q_w, k_w, v_w, o_w):
    x = np.asarray(x, dtype=np.float32)
    mask = np.triu(np.ones((128, 128), np.float32)).astype(NPBF)
    sel = np.zeros((2, 128), dtype=np.float32)
    sel[0, 0:64] = 1.0
    sel[1, 64:128] = 1.0
    sel = sel.astype(NPBF)
    xtb = [np.ascontiguousarray(x[b].T).astype(NPBF) for b in range(B)]

    def sbuf_layout(w_t, nchunk):
        # [nchunk*128, F] -> [128, nchunk, F] so each partition line is one
        # contiguous DMA segment
        f = w_t.shape[1]
        return np.ascontiguousarray(
            w_t.reshape(nchunk, 128, f).transpose(1, 0, 2)).astype(NPBF)

    in_maps = []
    for c in range(NCORES):
        b, hg = divmod(c, NCORES // B)
        sl = slice(hg * KD, (hg + 1) * KD)
        in_maps.append({
            "xt": xtb[b],
            "wqt": np.ascontiguousarray(
                sbuf_layout(np.asarray(q_w, np.float32)[sl, :].T, NDC)
                .reshape(128, NDC, NKK, 128).transpose(0, 2, 1, 3)),
            "wkt": sbuf_layout(np.asarray(k_w, np.float32)[sl, :].T, NDC),
            "wvt": sbuf_layout(np.asarray(v_w, np.float32)[sl, :].T, NDC),
            "wot": sbuf_layout(np.asarray(o_w, np.float32)[:, sl].T, NKK),
            "mask": mask,
            "sel": sel,
        })
    return in_maps


def combine_outputs(results):
    """results: list of 8 dicts with per-core partial y [S, D]."""
    per_b = NCORES // B
    ys = [np.asarray(results[c]["y"]).astype(np.float32) for c in range(NCORES)]
    out = np.stack(
        [sum(ys[b * per_b + i] for i in range(per_b)) for b in range(B)]
    )
    return np.ascontiguousarray(out, dtype=np.float32)


_PROGRAM = None


def kernel(x, q_proj_weight, k_proj_weight, v_proj_weight, o_proj_weight,
           **extra):
    global _PROGRAM
    if _PROGRAM is None:
        _PROGRAM = build_program()
    in_maps = make_in_maps(x, q_proj_weight, k_proj_weight, v_proj_weight,
                           o_proj_weight)
    res = run_bass_kernel_spmd(_PROGRAM, in_maps, list(range(NCORES)))
    return combine_outputs(res.results)


if __name__ == "__main__":
    nc = build_program()
    print("program built")


# revision 21
# speedup vs baseline: 1.1336x; 1.0297x over previous
"""Multi-head causal self-attention (B=2, S=2048, D=1024, H=16) on 8 trn2 cores.

Sharding: 2-way data-parallel over batch x 4-way tensor-parallel over heads.
Core c handles batch b=c//4 and heads [4*(c%4), 4*(c%4)+4).

Per-core device program (bf16 matmul operands, fp32 PSUM accumulation):
  1. QKV projections from host-pre-transposed x^T and W^T shards (all bf16).
     Q^T,K^T produced as [head-channel, token]; V as [token, channel] with a
     fused ones-column (softmax-denominator trick).
  2. Flash-style causal attention per (t-block 512, chunk of 2 heads):
     both heads' scores^T land in one [128, 2, 512] PSUM pair of banks, one
     batched exp (ScalarE, scale=1/8, bf16 out) serves both, diagonal tiles
     masked via a lower-triangle multiply on GpSimd, AV pairs trail two
     u-steps behind the scores so PE stays dense. AV accumulates into PSUM
     [65, 512] whose row 64 is the softmax denominator (ones column of V).
     Normalization after AV: denominator broadcast by a K=1 matmul,
     reciprocal_approx_fast, then one multiply into OT (bf16).
  3. o_proj partial product over this core's 256 v-dims, per t-block.
  The qkv projections for t-group g+1 and o_proj for block b-1 are
  interleaved into attention(b)'s instruction stream ("filler") so PE never
  drains while ScalarE runs exp.
  DMA: weights on the sync queue, x^T chunks on the scalar queue (parallel
  rings) so the first QK chain starts ~4us in instead of ~24us.
Host sums the 4 per-batch partials (the v-contraction all-reduce) and stacks.
"""

import numpy as np
from collections import deque
from contextlib import ExitStack

import ml_dtypes

import concourse.bass as bass
import concourse.bacc as bacc
import concourse.tile as tile
import concourse.mybir as mybir
from concourse.bass_utils import run_bass_kernel_spmd

F32 = mybir.dt.float32
BF16 = mybir.dt.bfloat16
EXP = mybir.ActivationFunctionType.Exp
NPBF = ml_dtypes.bfloat16

B, S, D = 2, 2048, 1024
H, HS = 16, 64
NCORES = 8
HPC = H // (NCORES // B)  # heads per core = 4
KD = HPC * HS             # per-core projected dims = 256
NKK = KD // 128           # head-dim partition chunks = 2
NDC = D // 128            # contraction chunks = 8
TB = 512                  # t-block width
NTB = S // TB             # 4
NUT = S // 128            # u-tiles = 16
SCALE = float(HS) ** -0.5


class Filler:
    """Queue of generators; each step(frac) emits fractional units of
    deferred PE work so filler spreads evenly over the attention loop."""

    def __init__(self, pace=1.0):
        self.gens = deque()
        self.pace = pace
        self.acc = 0.0

    def add(self, gen):
        self.gens.append(gen)

    def _one(self):
        while self.gens:
            try:
                next(self.gens[0])
                return True
            except StopIteration:
                self.gens.popleft()
        return False

    def step(self, n=None):
        self.acc += self.pace if n is None else n
        while self.acc >= 1.0:
            self.acc -= 1.0
            if not self._one():
                return

    def drain(self):
        while self._one():
            pass


def build_program():
    nc = bacc.Bacc("TRN2", target_bir_lowering=False, debug=False)
    # weights arrive pre-laid-out for SBUF ([partition, chunk, free]) so the
    # DMA is one contiguous multi-KB segment per partition line.
    xt = nc.dram_tensor("xt", [D, S], BF16, kind="ExternalInput").ap()
    wqt = nc.dram_tensor("wqt", [128, NDC, KD], BF16, kind="ExternalInput").ap()
    wkt = nc.dram_tensor("wkt", [128, NDC, KD], BF16, kind="ExternalInput").ap()
    wvt = nc.dram_tensor("wvt", [128, NDC, KD], BF16, kind="ExternalInput").ap()
    wot = nc.dram_tensor("wot", [128, NKK, D], BF16, kind="ExternalInput").ap()
    maskd = nc.dram_tensor("mask", [128, 128], BF16, kind="ExternalInput").ap()
    seld = nc.dram_tensor("sel", [2, 128], BF16, kind="ExternalInput").ap()
    y = nc.dram_tensor("y", [S, D], BF16, kind="ExternalOutput").ap()

    with tile.TileContext(nc) as tc, ExitStack() as ctx:
        wpool = ctx.enter_context(tc.tile_pool(name="w", bufs=1))
        big = ctx.enter_context(tc.tile_pool(name="big", bufs=1))
        xtg_pool = ctx.enter_context(tc.tile_pool(name="xtg", bufs=32))
        e_pool = ctx.enter_context(tc.tile_pool(name="expS", bufs=6))
        sm_pool = ctx.enter_context(tc.tile_pool(name="small", bufs=2))
        ypool = ctx.enter_context(tc.tile_pool(name="yout", bufs=3))
        psS = ctx.enter_context(tc.tile_pool(name="psS", bufs=2, space="PSUM"))
        psO = ctx.enter_context(tc.tile_pool(name="psO", bufs=2, space="PSUM"))
        psM = ctx.enter_context(tc.tile_pool(name="psM", bufs=2, space="PSUM"))

        # --- persistent SBUF tensors ---
        wq_sb = wpool.tile([128, NDC, KD], BF16)
        wk_sb = wpool.tile([128, NDC, KD], BF16)
        wv_sb = wpool.tile([128, NDC, KD], BF16)
        wo_sb = wpool.tile([128, NKK, D], BF16)
        mask_sb = wpool.tile([128, 128], BF16)
        selA_sb = wpool.tile([1, 128], BF16)
        selB_sb = wpool.tile([1, 128], BF16)
        QT = big.tile([128, NKK, S], BF16)   # [channel(2 heads), kk, token]
        KT = big.tile([128, NKK, S], BF16)
        VA = big.tile([128, HPC, NUT, HS + 1], BF16)  # [tok, head, utile, ch|1]
        OT = big.tile([128, NKK, S], BF16)   # normalized attention out^T

        # weights on the sync DMA queue (already in SBUF layout)
        nc.sync.dma_start(wq_sb[:], wqt)
        nc.sync.dma_start(wk_sb[:], wkt)
        nc.sync.dma_start(wv_sb[:], wvt)
        nc.sync.dma_start(mask_sb[:], maskd)
        nc.sync.dma_start(selA_sb[:], seld[0:1, :])
        nc.sync.dma_start(selB_sb[:], seld[1:2, :])
        nc.sync.dma_start(wo_sb[:], wot)
        nc.gpsimd.memset(VA[:, :, :, HS], 1.0)

        # x^T on the scalar DMA queue, tg-major so t-group 0 lands first;
        # batched 4 c-chunks per DMA to keep ScalarE's issue cost small.
        xts = [[None] * NTB for _ in range(NDC)]
        for tg in range(NTB):
            for h in range(2):
                t = xtg_pool.tile([128, 4, TB], BF16, tag="xtg",
                                  name=f"xt{h}_{tg}")
                nc.scalar.dma_start(
                    t[:],
                    xt[512 * h:512 * (h + 1),
                       TB * tg:TB * (tg + 1)].rearrange(
                           "(c p) t -> p c t", p=128),
                )
                for c4 in range(4):
                    xts[4 * h + c4][tg] = t[:, c4, :]

        def qkv_gen(tg):
            """Projections for t-group tg, yielding every ~4 matmuls."""
            for w_sb, dst in ((wq_sb, QT), (wk_sb, KT)):
                for kk in range(NKK):
                    ps = psM.tile([128, TB], F32, tag="m512", name="qk_ps")
                    for c in range(NDC):
                        nc.tensor.matmul(
                            ps[:],
                            w_sb[:, c, 128 * kk:128 * (kk + 1)],
                            xts[c][tg][:],
                            start=(c == 0), stop=(c == NDC - 1),
                        )
                        if c % 4 == 3:
                            yield
                    nc.vector.tensor_copy(dst[:, kk, TB * tg:TB * (tg + 1)], ps[:])
                    yield
            for tt in range(TB // 128):
                ps = psM.tile([128, TB], F32, tag="m512", name="v_ps")
                for c in range(NDC):
                    nc.tensor.matmul(
                        ps[:, 0:KD],
                        xts[c][tg][:, 128 * tt:128 * (tt + 1)],
                        wv_sb[:, c, :],
                        start=(c == 0), stop=(c == NDC - 1),
                    )
                    if c % 4 == 3:
                        yield
                ut = (TB // 128) * tg + tt
                nc.vector.tensor_copy(
                    VA[:, :, ut, 0:HS],
                    ps[:, 0:KD].rearrange("p (h c) -> p h c", c=HS),
                )
                yield

        def o_proj_gen(tb, split_engines=False):
            for i in range(4 * tb, 4 * tb + 4):
                for j in range(D // 512):
                    ps = psM.tile([128, TB], F32, tag="m512", name="yps")
                    for vc in range(NKK):
                        nc.tensor.matmul(
                            ps[:],
                            OT[:, vc, 128 * i:128 * (i + 1)],
                            wo_sb[:, vc, 512 * j:512 * (j + 1)],
                            start=(vc == 0), stop=(vc == NKK - 1),
                        )
                    yt = ypool.tile([128, TB], BF16, tag="yt")
                    # tail block: drain copies on both engines in parallel
                    if split_engines and (i + j) % 2 == 1:
                        nc.scalar.copy(yt[:], ps[:])
                        nc.scalar.dma_start(
                            y[128 * i:128 * (i + 1), 512 * j:512 * (j + 1)],
                            yt[:])
                    else:
                        nc.vector.tensor_copy(yt[:], ps[:])
                        nc.sync.dma_start(
                            y[128 * i:128 * (i + 1), 512 * j:512 * (j + 1)],
                            yt[:])
                    yield

        def attention(tb, filler):
            """Causal attention for t-block tb, heads paired per chunk, AV
            trailing two u-steps behind scores. filler work is pulled in
            after each u-step so PE stays dense while ScalarE runs exp."""
            nut = 4 * tb + 4
            for hp in range(NKK):
                Os = [psO.tile([HS + 1, TB], F32, tag="av", name=f"O{g}")
                      for g in range(2)]

                def av_pair(pes, ptoff, pk, stop):
                    for g in range(2):
                        nc.tensor.matmul(
                            Os[g][:, ptoff:],
                            VA[:, 2 * hp + g, pk, :],
                            pes[:, g, ptoff:],
                            start=(pk == 0), stop=stop,
                        )

                pend = []
                for k in range(nut):
                    toff = max(0, 128 * (k - 4 * tb))
                    sp = psS.tile([128, 2, TB], F32, tag="sp")
                    for g in range(2):
                        nc.tensor.matmul(
                            sp[:, g, toff:],
                            KT[64 * g:64 * g + 64, hp, 128 * k:128 * (k + 1)],
                            QT[64 * g:64 * g + 64, hp,
                               TB * tb + toff:TB * (tb + 1)],
                            start=True, stop=True,
                        )
                    es = e_pool.tile([128, 2, TB], BF16, tag="expS")
                    nc.scalar.activation(es[:, :, toff:], sp[:, :, toff:], EXP,
                                         scale=SCALE)
                    if k >= 4 * tb:  # diagonal: zero the u>t triangle
                        for g in range(2):
                            nc.gpsimd.tensor_mul(
                                es[:, g, toff:toff + 128],
                                es[:, g, toff:toff + 128], mask_sb[:]
                            )
                    pend.append((es, toff, k))
                    if len(pend) > 2:
                        av_pair(*pend.pop(0), stop=False)
                    filler.step()
                for i, p in enumerate(pend):
                    av_pair(*p, stop=(i == len(pend) - 1))
                dens = [sm_pool.tile([1, TB], BF16, tag=f"den{g}",
                                     name=f"den{g}") for g in range(2)]
                for g in range(2):
                    nc.vector.tensor_copy(dens[g][:], Os[g][HS:HS + 1, :])
                bc_ps = psM.tile([128, TB], F32, tag="m512", name="bc_ps")
                nc.tensor.matmul(bc_ps[:], selA_sb[:], dens[0][:],
                                 start=True, stop=False)
                nc.tensor.matmul(bc_ps[:], selB_sb[:], dens[1][:],
                                 start=False, stop=True)
                # keep PE fed while the norm chain drains on DVE
                filler.step(2)
                bc = sm_pool.tile([128, TB], F32, tag="bc_sb")
                nc.vector.reciprocal_approx_fast(bc[:], bc_ps[:])
                for g in range(2):
                    ro = 64 * g
                    nc.vector.tensor_mul(
                        OT[ro:ro + 64, hp, TB * tb:TB * (tb + 1)],
                        Os[g][0:HS, :], bc[ro:ro + 64, :]
                    )
                filler.step(2)

        boot = Filler()
        boot.add(qkv_gen(0))
        boot.drain()
        # filler content per attention block, paced to spread over its
        # k-steps: o_proj work is deferred toward the late (exp-paced) blocks.
        plans = [
            (1.5, [qkv_gen(1)]),
            (0.75, [qkv_gen(2)]),
            (0.5, [qkv_gen(3)]),
            (0.375, [o_proj_gen(0), o_proj_gen(1), o_proj_gen(2)]),
        ]
        for tb in range(NTB):
            pace, gens = plans[tb]
            f = Filler(pace)
            for g in gens:
                f.add(g)
            attention(tb, f)
            f.drain()
        tail = Filler()
        tail.add(o_proj_gen(NTB - 1, split_engines=True))
        tail.drain()

    nc.compile()
    return nc


def make_in_maps(x, q_w, k_w, v_w, o_w):
    x = np.asarray(x, dtype=np.float32)
    mask = np.triu(np.ones((128, 128), np.float32)).astype(NPBF)
    sel = np.zeros((2, 128), dtype=np.float32)
    sel[0, 0:64] = 1.0
    sel[1, 64:128] = 1.0
    sel = sel.astype(NPBF)
    xtb = [np.ascontiguousarray(x[b].T).astype(NPBF) for b in range(B)]

    def sbuf_layout(w_t, nchunk):
        # [nchunk*128, F] -> [128, nchunk, F] so each partition line is one
        # contiguous DMA segment
        f = w_t.shape[1]
        return np.ascontiguousarray(
            w_t.reshape(nchunk, 128, f).transpose(1, 0, 2)).astype(NPBF)

    in_maps = []
    for c in range(NCORES):
        b, hg = divmod(c, NCORES // B)
        sl = slice(hg * KD, (hg + 1) * KD)
        in_maps.append({
            "xt": xtb[b],
            "wqt": sbuf_layout(np.asarray(q_w, np.float32)[sl, :].T, NDC),
            "wkt": sbuf_layout(np.asarray(k_w, np.float32)[sl, :].T, NDC),
            "wvt": sbuf_layout(np.asarray(v_w, np.float32)[sl, :].T, NDC),
            "wot": sbuf_layout(np.asarray(o_w, np.float32)[:, sl].T, NKK),
            "mask": mask,
            "sel": sel,
        })
    return in_maps


def combine_outputs(results):
    """results: list of 8 dicts with per-core partial y [S, D]."""
    per_b = NCORES // B
    ys = [np.asarray(results[c]["y"]).astype(np.float32) for c in range(NCORES)]
    out = np.stack(
        [sum(ys[b * per_b + i] for i in range(per_b)) for b in range(B)]
    )
    return np.ascontiguousarray(out, dtype=np.float32)


_PROGRAM = None


def kernel(x, q_proj_weight, k_proj_weight, v_proj_weight, o_proj_weight,
           **extra):
    global _PROGRAM
    if _PROGRAM is None:
        _PROGRAM = build_program()
    in_maps = make_in_maps(x, q_proj_weight, k_proj_weight, v_proj_weight,
                           o_proj_weight)
    res = run_bass_kernel_spmd(_PROGRAM, in_maps, list(range(NCORES)))
    return combine_outputs(res.results)


if __name__ == "__main__":
    nc = build_program()
    print("program built")
